# revision 1
# baseline (speedup 1.0000x reference)
"""MoE FeedForward kernel for 8 Trainium2 NeuronCores.

Strategy (expert-parallel dispatch-by-assignment, per the sharding hint):
  - Host computes the gate (logits -> top-2 -> assign = max index, w = softmax sum)
    on jax-CPU for bit-parity with the reference's routing decisions.
  - Tokens are sorted by assigned expert, padded to 128-token tiles, and the
    tiles are packed into 16 "slots" (2 per core: s1 + s2 tiles).  Each slot
    serves exactly one expert, so a core touches at most 2 experts' weights.
  - The device kernel (SPMD, same program on all 8 cores) runs, per 128-token
    tile: x @ W1.T (bf16 matmul, fp32 accum) -> +b1 -> LayerNorm (stats fused
    into DVE/ACT passes) -> exact-erf GELU (normalize fused into the ACT pass)
    -> PE transpose -> h @ W2.T -> y*alpha + (x*w + b2*alpha)  (epilogue fused
    into one DVE pass; the x/b2 term is precomputed on host).
  - Host scatters the per-token rows back to their original positions.
"""

import math
import os

import numpy as np
import ml_dtypes

os.environ.setdefault("MYCRO_LOCAL_CACHE", "1")

B, S, D, F, E = 4, 2048, 1024, 2048, 8
T = B * S
NCORES = 8
PTILE = 128  # tokens per tile
LN_EPS = 1e-5
BF16 = ml_dtypes.bfloat16

_PROG_CACHE = {}
LAST_RESULT = None  # BassKernelResults of the most recent run (for test harness)
LAST_CALL = None  # (nc, in_maps) of the most recent run (for test harness)


def _split_multi_waits(nc, mybir):
    """TPB engine instructions encode exactly ONE semaphore wait
    (NEURON_ISA_TPB_EVENTS has a single wait slot); walrus codegen rejects
    instructions with more.  Split extra waits onto preceding same-engine
    NoOps (engine queues are FIFO, so gating a NoOp gates the instruction)."""
    skip = {"UnconditionalBranch", "ConditionalBranch", "Call", "EventSemaphore"}
    work = []
    for fn in nc.m.functions:
        for blk in fn.blocks:
            for ins in blk.instructions:
                si = ins.sync_info
                waits = list(si.on_wait) if si is not None and si.on_wait else []
                if len(waits) > 1 and str(ins.opcode) not in skip:
                    work.append((ins, waits, si))
    if not work:
        return
    created = {}
    for ins, waits, si in work:
        nops = []
        for w in waits[:-1]:
            bi = nc.engines[ins.engine].nop(nofuse=True)
            ni = bi.ins
            ni.sync_info = mybir.SyncInfo(on_wait=[w], on_update=[])
            nops.append(ni)
        ins.sync_info = mybir.SyncInfo(
            on_wait=[waits[-1]],
            on_update=list(si.on_update) if si.on_update else [],
        )
        created[str(ins.name)] = nops
    nop_names = {str(n.name) for ns in created.values() for n in ns}
    for fn in nc.m.functions:
        for blk in fn.blocks:
            new_list = []
            for ins in blk.instructions:
                nm = str(ins.name)
                if nm in nop_names:
                    continue  # strip from appended position
                if nm in created:
                    new_list.extend(created[nm])
                new_list.append(ins)
            blk.instructions = new_list


def _build_program(tpc, s1, s2, general_ln):
    """Build the SPMD Bass/Tile program: tpc tiles per core, split s1/s2 across
    the two weight slots."""
    from contextlib import ExitStack

    import concourse.bass as bass
    import concourse.mybir as mybir
    import concourse.tile as tile
    from concourse.masks import make_identity

    dt = mybir.dt
    Alu = mybir.AluOpType
    Act = mybir.ActivationFunctionType

    nc = bass.Bass()
    xtt = nc.declare_dram_parameter("xtt", [tpc, 128, D], dt.bfloat16, False)
    xbp_d = nc.declare_dram_parameter("xb", [tpc, 128, D], dt.bfloat16, False)
    alp_d = nc.declare_dram_parameter("alp", [tpc, 128, 1], dt.float32, False)
    w1_d = nc.declare_dram_parameter("w1", [2, 128, 8 * F], dt.bfloat16, False)
    w2_d = nc.declare_dram_parameter("w2", [2, 128, 16 * D], dt.bfloat16, False)
    b1_d = nc.declare_dram_parameter("b1r", [2, 128, F], dt.bfloat16, False)
    if general_ln:
        g_d = nc.declare_dram_parameter("gr", [2, 128, F], dt.bfloat16, False)
        bb_d = nc.declare_dram_parameter("br", [2, 128, F], dt.bfloat16, False)
    out_d = nc.declare_dram_parameter("out", [tpc, 128, D], dt.float32, True)

    with ExitStack() as ctx:
        tc = ctx.enter_context(tile.TileContext(nc))
        wp1 = ctx.enter_context(tc.tile_pool(name="w1p", bufs=2))
        wp2 = ctx.enter_context(tc.tile_pool(name="w2p", bufs=2))
        bp = ctx.enter_context(tc.tile_pool(name="b1p", bufs=2))
        xp = ctx.enter_context(tc.tile_pool(name="xp", bufs=3))
        xbp = ctx.enter_context(tc.tile_pool(name="xbp", bufs=3))
        alp = ctx.enter_context(tc.tile_pool(name="alp", bufs=3))
        hp = ctx.enter_context(tc.tile_pool(name="hp", bufs=2))
        h2p = ctx.enter_context(tc.tile_pool(name="h2p", bufs=4))
        hTp = ctx.enter_context(tc.tile_pool(name="hTp", bufs=2))
        fpool = ctx.enter_context(tc.tile_pool(name="fp", bufs=3))
        sp = ctx.enter_context(tc.tile_pool(name="sp", bufs=3))
        cp = ctx.enter_context(tc.tile_pool(name="cp", bufs=1))
        ph = ctx.enter_context(tc.tile_pool(name="ph", bufs=1, space="PSUM"))
        pt = ctx.enter_context(tc.tile_pool(name="pt", bufs=2, space="PSUM"))
        py = ctx.enter_context(tc.tile_pool(name="py", bufs=1, space="PSUM"))
        if general_ln:
            gp = ctx.enter_context(tc.tile_pool(name="gp", bufs=2))
            hnp = ctx.enter_context(tc.tile_pool(name="hnp", bufs=2))

        ident = cp.tile([128, 128], dt.bfloat16, tag="ident")
        make_identity(nc, ident)
        epst = cp.tile([128, 1], dt.float32, tag="eps")
        nc.gpsimd.memset(epst, LN_EPS)
        tp = ctx.enter_context(tc.tile_pool(name="tp", bufs=4))

        def dve_touch(ap):
            # Absorb a DMA-completion wait into the DVE vector clock so the
            # next real DVE consumer doesn't exceed its sync-wait slot limit.
            t = tp.tile([128, 1], dt.float32, tag="touch")
            nc.vector.tensor_copy(t, ap[:, 0:1])

        for slot in range(2):
            ntl = s1 if slot == 0 else s2
            # biases first on the lightly-loaded sync ring
            b1t = bp.tile([128, F], dt.bfloat16, tag="b1")
            nc.sync.dma_start(b1t, b1_d[slot])
            # weight chunks alternate between two DMA rings (Pool + ACT) so
            # they transfer in parallel, and the first matmul can start as
            # soon as its first K-chunk lands
            w1t = wp1.tile([128, 8 * F], dt.bfloat16, tag="w1")
            for q in range(4):
                eng = nc.gpsimd if q % 2 == 0 else nc.scalar
                eng.dma_start(w1t[:, q * 2 * F:(q + 1) * 2 * F],
                              w1_d[slot][:, q * 2 * F:(q + 1) * 2 * F])
            w2t = wp2.tile([128, 16 * D], dt.bfloat16, tag="w2")
            for q in range(4):
                eng = nc.gpsimd if q % 2 == 1 else nc.scalar
                eng.dma_start(w2t[:, q * 4 * D:(q + 1) * 4 * D],
                              w2_d[slot][:, q * 4 * D:(q + 1) * 4 * D])
            if general_ln:
                gt = gp.tile([128, F], dt.bfloat16, tag="g")
                nc.gpsimd.dma_start(gt, g_d[slot])
                bbt = gp.tile([128, F], dt.bfloat16, tag="bb")
                nc.gpsimd.dma_start(bbt, bb_d[slot])

            for tl in range(ntl):
                tg = (0 if slot == 0 else s1) + tl

                xt = xp.tile([128, 8 * 128], dt.bfloat16, tag="xt")
                nc.sync.dma_start(xt, xtt[tg])
                xbt = xbp.tile([128, D], dt.bfloat16, tag="xb")
                nc.sync.dma_start(xbt, xbp_d[tg])
                al = alp.tile([128, 1], dt.float32, tag="al")
                nc.sync.dma_start(al, alp_d[tg])

                # ---- matmul1: h[tok, F] = x @ W1.T  (accumulate over D) ----
                hps = ph.tile([128, F], dt.float32, tag="hps")
                for d in range(8):
                    lhsT = xt[:, d * 128:(d + 1) * 128]
                    for fb in range(4):
                        nc.tensor.matmul(
                            hps[:, fb * 512:(fb + 1) * 512],
                            lhsT=lhsT,
                            rhs=w1t[:, d * F + fb * 512: d * F + fb * 512 + 512],
                            start=(d == 0),
                            stop=(d == 7),
                        )

                # ---- evacuate + bias + row-sum (one DVE pass) ----
                h1 = hp.tile([128, F], dt.float32, tag="h1")
                s1t = sp.tile([128, 1], dt.float32, tag="s1")
                nc.vector.scalar_tensor_tensor(
                    out=h1, in0=hps, scalar=0.0, in1=b1t,
                    op0=Alu.add, op1=Alu.add, accum_out=s1t,
                )
                # ---- sum of squares (ACT pass; large out is a throwaway) ----
                junk = h2p.tile([128, F], dt.bfloat16, tag="h2")
                s2t = sp.tile([128, 1], dt.float32, tag="s2")
                nc.scalar.activation(out=junk, in_=h1, func=Act.Square, accum_out=s2t)
                # ---- LN scalars ----
                ss = sp.tile([128, 1], dt.float32, tag="ss")
                nc.vector.tensor_tensor(out=ss, in0=s1t, in1=s1t, op=Alu.mult)
                varf = sp.tile([128, 1], dt.float32, tag="varf")
                nc.vector.scalar_tensor_tensor(
                    out=varf, in0=ss, scalar=-1.0 / F, in1=s2t,
                    op0=Alu.mult, op1=Alu.add,
                )
                sq = sp.tile([128, 1], dt.float32, tag="sq")
                nc.scalar.activation(out=sq, in_=varf, func=Act.Sqrt,
                                     scale=1.0 / F, bias=epst)
                rstd = sp.tile([128, 1], dt.float32, tag="rstd")
                nc.vector.reciprocal(rstd, sq)
                bg = sp.tile([128, 1], dt.float32, tag="bg")
                nc.vector.scalar_tensor_tensor(
                    out=bg, in0=s1t, scalar=-1.0 / F, in1=rstd,
                    op0=Alu.mult, op1=Alu.mult,
                )
                # ---- normalize + gelu (fused into ACT unless general ln) ----
                h2 = h2p.tile([128, F], dt.bfloat16, tag="h2")
                if not general_ln:
                    nc.scalar.activation(out=h2, in_=h1, func=Act.Gelu,
                                         scale=rstd, bias=bg)
                else:
                    hn = hnp.tile([128, F], dt.float32, tag="hn")
                    nc.scalar.activation(out=hn, in_=h1, func=Act.Identity,
                                         scale=rstd, bias=bg)
                    hn2 = hnp.tile([128, F], dt.float32, tag="hn2")
                    nc.vector.scalar_tensor_tensor(
                        out=hn2, in0=hn, scalar=0.0, in1=gt,
                        op0=Alu.add, op1=Alu.mult,
                    )
                    hn3 = hnp.tile([128, F], dt.float32, tag="hn")
                    nc.vector.scalar_tensor_tensor(
                        out=hn3, in0=hn2, scalar=0.0, in1=bbt,
                        op0=Alu.add, op1=Alu.add,
                    )
                    nc.scalar.activation(out=h2, in_=hn3, func=Act.Gelu)

                # ---- transpose h2 -> hT (PE), evacuate to SBUF bf16 ----
                hT = hTp.tile([128, F], dt.bfloat16, tag="hT")
                for f in range(16):
                    ptile = pt.tile([128, 128], dt.bfloat16, tag="pt")
                    nc.tensor.transpose(ptile, h2[:, f * 128:(f + 1) * 128], ident)
                    if f % 2 == 0:
                        nc.vector.tensor_copy(hT[:, f * 128:(f + 1) * 128], ptile)
                    else:
                        nc.scalar.copy(hT[:, f * 128:(f + 1) * 128], ptile)

                # ---- matmul2: y[tok, D] = h @ W2.T (accumulate over F) ----
                yps = py.tile([128, D], dt.float32, tag="yps")
                for f in range(16):
                    lhsT = hT[:, f * 128:(f + 1) * 128]
                    for db in range(2):
                        nc.tensor.matmul(
                            yps[:, db * 512:(db + 1) * 512],
                            lhsT=lhsT,
                            rhs=w2t[:, f * D + db * 512: f * D + db * 512 + 512],
                            start=(f == 0),
                            stop=(f == 15),
                        )

                # ---- epilogue: out = y*alpha + xb  (one DVE pass) ----
                fin = fpool.tile([128, D], dt.float32, tag="fin")
                nc.vector.scalar_tensor_tensor(
                    out=fin, in0=yps, scalar=al, in1=xbt,
                    op0=Alu.mult, op1=Alu.add,
                )
                nc.sync.dma_start(out_d[tg], fin)

    if os.environ.get("NO_WAITSPLIT") != "1":
        _split_multi_waits(nc, mybir)
    return nc


def _gate_host(xr, Wg, bg):
    """Replicate the reference's routing math on jax-CPU for bit-parity."""
    import jax
    import jax.numpy as jnp

    cpu = jax.devices("cpu")[0]
    with jax.default_device(cpu):
        xj = jnp.asarray(xr)
        logits = xj @ jnp.asarray(Wg).T + jnp.asarray(bg)
        top_v, top_i = jax.lax.top_k(logits, 2)
        w = jnp.sum(jax.nn.softmax(top_v, axis=-1), axis=-1)
        assign = jnp.max(top_i, axis=-1)
        return np.asarray(assign), np.asarray(w, dtype=np.float32)


def _pack_slots(counts):
    """Pack per-expert tile demands into 16 single-expert slots (8 of size s1,
    8 of size s2, s1+s2 = tpc) minimizing tpc via exact DP over how many
    s1-slots (a) and s2-slots (b) each expert takes.
    Returns (tpc, s1, s2, core_slots): core i runs core_slots[i] = (slotA of
    size s1, slotB of size s2), each {expert, size, nreal}."""
    demands = {e: int(math.ceil(c / PTILE)) for e, c in enumerate(counts) if c > 0}
    experts = sorted(demands, key=lambda k: -demands[k])
    total = sum(demands.values())
    tpc = max(2, math.ceil(total / NCORES))
    while True:
        s1 = math.ceil(tpc / 2)
        s2 = tpc - s1
        # per-expert pareto options (a s1-slots, b s2-slots)
        opts = []
        for e in experts:
            d = demands[e]
            o = []
            for a in range(9):
                for b in range(9):
                    if a + b == 0:
                        continue
                    if a * s1 + b * s2 >= d:
                        if not any(a2 <= a and b2 <= b for a2, b2 in o):
                            o.append((a, b))
            o = [(a, b) for a, b in o
                 if not any((a2 <= a and b2 <= b and (a2, b2) != (a, b))
                            for a2, b2 in o)]
            opts.append(o)
        # DP over (fives_used, fours_used)
        states = {(0, 0): []}
        for o in opts:
            nxt = {}
            for (ua, ub), path in states.items():
                for a, b in o:
                    k = (ua + a, ub + b)
                    if k[0] <= 8 and k[1] <= 8 and k not in nxt:
                        nxt[k] = path + [(a, b)]
            states = nxt
            if not states:
                break
        if states:
            choice = next(iter(states.values()))
            break
        tpc += 1
    g1, g2 = [], []
    for e, (a, b) in zip(experts, choice):
        rem = demands[e]
        for _ in range(a):
            g1.append({"expert": e, "size": s1, "nreal": min(rem, s1)})
            rem -= min(rem, s1)
        for _ in range(b):
            g2.append({"expert": e, "size": s2, "nreal": min(rem, s2)})
            rem -= min(rem, s2)
        assert rem == 0
    big_e = experts[0]
    while len(g1) < 8:
        g1.append({"expert": big_e, "size": s1, "nreal": 0})
    while len(g2) < 8:
        g2.append({"expert": big_e, "size": s2, "nreal": 0})
    assert len(g1) == 8 and len(g2) == 8
    # pair heavier s1 slots with lighter s2 slots (cosmetic; compute is fixed)
    return tpc, s1, s2, list(zip(g1, g2[::-1]))


def kernel(x, Wg, bg, W1, b1, ln_g, ln_b, W2, b2, res_scale):
    global LAST_RESULT
    x = np.asarray(x, dtype=np.float32)
    Wg = np.asarray(Wg, dtype=np.float32)
    bg = np.asarray(bg, dtype=np.float32)
    W1 = np.asarray(W1, dtype=np.float32)
    b1 = np.asarray(b1, dtype=np.float32)
    ln_g = np.asarray(ln_g, dtype=np.float32)
    ln_b = np.asarray(ln_b, dtype=np.float32)
    W2 = np.asarray(W2, dtype=np.float32)
    b2 = np.asarray(b2, dtype=np.float32)
    res_scale = np.asarray(res_scale, dtype=np.float32)

    xr = x.reshape(T, D)
    assign, w = _gate_host(xr, Wg, bg)

    counts = np.bincount(assign, minlength=E)
    order = np.argsort(assign, kind="stable")
    tpc, s1, s2, core_slots = _pack_slots(counts)
    general_ln = not (np.all(ln_g == 1.0) and np.all(ln_b == 0.0))

    # per-expert padded tile arrays (token ids) + validity
    starts = np.zeros(E + 1, np.int64)
    np.cumsum(counts, out=starts[1:])
    exp_tiles = {}
    for e in range(E):
        c = int(counts[e])
        if c == 0:
            continue
        toks = order[starts[e]:starts[e] + c]
        ntl = math.ceil(c / PTILE)
        padded = np.concatenate([toks, np.repeat(toks[-1], ntl * PTILE - c)])
        valid = np.zeros(ntl * PTILE, bool)
        valid[:c] = True
        exp_tiles[e] = (padded.reshape(ntl, PTILE), valid.reshape(ntl, PTILE))
    cursor = {e: 0 for e in exp_tiles}

    # pre-pack weights for the active experts
    used = sorted({s["expert"] for pair in core_slots for s in pair})
    W1P, W2P, B1R, GR, BR = {}, {}, {}, {}, {}
    for e in used:
        W1P[e] = np.ascontiguousarray(
            W1[e].T.reshape(8, 128, F).transpose(1, 0, 2).reshape(128, 8 * F)
        ).astype(BF16)
        W2P[e] = np.ascontiguousarray(
            W2[e].T.reshape(16, 128, D).transpose(1, 0, 2).reshape(128, 16 * D)
        ).astype(BF16)
        B1R[e] = np.broadcast_to(b1[e], (128, F)).astype(BF16)
        if general_ln:
            GR[e] = np.broadcast_to(ln_g[e], (128, F)).astype(BF16)
            BR[e] = np.broadcast_to(ln_b[e], (128, F)).astype(BF16)

    in_maps = []
    scatter = []  # per core: (token_ids, valid)
    for slot_a, slot_b in core_slots:
        tok_ids = np.zeros((tpc, PTILE), np.int64)
        valid = np.zeros((tpc, PTILE), bool)
        e_tile = np.zeros(tpc, np.int64)
        ti = 0
        for slot, size in ((slot_a, s1), (slot_b, s2)):
            e = slot["expert"]
            tiles, vmask = exp_tiles.get(e, (None, None))
            for k in range(size):
                if k < slot["nreal"]:
                    idx = cursor[e]
                    cursor[e] += 1
                    tok_ids[ti] = tiles[idx]
                    valid[ti] = vmask[idx]
                else:
                    tok_ids[ti] = tiles[0] if tiles is not None else 0
                    valid[ti] = False
                e_tile[ti] = e
                ti += 1
        ids = tok_ids.reshape(-1)
        e_row = np.repeat(e_tile, PTILE)
        xg = xr[ids]  # [tpc*128, D]
        w_row = w[ids]
        alpha_row = res_scale[e_row] * w_row
        xb_rows = xg * w_row[:, None] + alpha_row[:, None] * b2[e_row]
        xtt = (
            xg.reshape(tpc, PTILE, 8, 128)
            .transpose(0, 3, 2, 1)
            .reshape(tpc, 128, 8 * 128)
        ).astype(BF16)
        im = {
            "xtt": np.ascontiguousarray(xtt),
            "xb": xb_rows.reshape(tpc, PTILE, D).astype(BF16),
            "alp": alpha_row.reshape(tpc, PTILE, 1).astype(np.float32),
            "w1": np.stack([W1P[slot_a["expert"]], W1P[slot_b["expert"]]]),
            "w2": np.stack([W2P[slot_a["expert"]], W2P[slot_b["expert"]]]),
            "b1r": np.stack([B1R[slot_a["expert"]], B1R[slot_b["expert"]]]),
        }
        if general_ln:
            im["gr"] = np.stack([GR[slot_a["expert"]], GR[slot_b["expert"]]])
            im["br"] = np.stack([BR[slot_a["expert"]], BR[slot_b["expert"]]])
        in_maps.append(im)
        scatter.append((ids, valid.reshape(-1)))

    global _LAST_SCATTER
    _LAST_SCATTER = scatter

    key = (tpc, s1, s2, general_ln)
    if key not in _PROG_CACHE:
        _PROG_CACHE[key] = _build_program(*key)
    nc = _PROG_CACHE[key]

    from concourse.bass_utils import run_bass_kernel_spmd

    global LAST_CALL
    LAST_CALL = (nc, in_maps)
    res = run_bass_kernel_spmd(nc, in_maps, core_ids=list(range(NCORES)))
    LAST_RESULT = res

    out = np.zeros((T, D), np.float32)
    covered = 0
    for core in range(NCORES):
        rows = np.asarray(res.results[core]["out"]).reshape(tpc * PTILE, D)
        ids, valid = scatter[core]
        out[ids[valid]] = rows[valid]
        covered += int(valid.sum())
    assert covered == T, f"coverage {covered} != {T}"
    return out.reshape(B, S, D)

